# revision 39
# baseline (speedup 1.0000x reference)
"""DenseCaps routing kernel for 8x Trainium2 NeuronCores.

Shapes (hardcoded): inputs (16, 2048, 16) f32, w (2048, 16, 64, 32) f32.
Sharding: ch_i (2048) split 8 ways -> 256 i's per core. Each core computes
u[b, i_loc, j, m] via a block-diagonal stationary matmul streaming its w
shard once (bf16), keeps u resident in SBUF (bf16), runs the 3-iteration
dynamic routing locally, and AllReduces the small s[b, j, m] partial sums.

Free-dim layout is "paired": f = jp*64 + m*2 + j2 with j = 2*jp + j2.
This makes the c-weighting of u a single 2x-mode tensor_tensor with a
stride-0 broadcast AP (no materialized broadcast), and the m-reduction a
tree of 2x-mode adds instead of a 1x tensor_reduce. u is stored as 16
double-width tiles (two production tiles each) so the routing rounds run
half as many vector-engine instructions at twice the width.

The s partials are AllReduced in two halves per round so the first
collective overlaps the second half of the tile loop; a tiny warmup
AllReduce at kernel start absorbs the collective-stack init cost. The
softmax normalization (64/Z) is folded into the per-tile s-matmul
stationary, so the c-weighting multiply uses raw exp values.

Output v (16, 64, 32) f32 is identical on all cores; core 0's is returned.
"""

import sys
from contextlib import ExitStack

import numpy as np

sys.path.insert(0, "/opt/trn_rl_repo")

import concourse.bass as bass
import concourse.bacc as bacc
import concourse.tile as tile
from concourse import mybir
from concourse.bass_utils import run_bass_kernel_spmd

F32 = mybir.dt.float32
BF16 = mybir.dt.bfloat16

B = 16
CH_I = 2048
N_I = 16
CH_J = 64
N_J = 32
JM = CH_J * N_J  # 2048
JP = CH_J // 2  # 32 j-pairs
N_CORES = 8
I_LOC = CH_I // N_CORES  # 256
N_T = I_LOC // 8  # 32 production tiles, 8 i's each
N_P = N_T // 2  # 16 double tiles
H_T = N_T // 2
EPS = 1e-7

_CACHE = {}


def _build_program(trace=False):
    nc = bacc.Bacc("TRN2", target_bir_lowering=False, debug=False,
                   num_devices=N_CORES)

    xblk_d = nc.dram_tensor("xblk", [128, N_T * 128], BF16,
                            kind="ExternalInput")
    w_d = nc.dram_tensor("w", [I_LOC * N_I, JM], BF16, kind="ExternalInput")
    s16_d = nc.dram_tensor("s16", [128, 16], BF16, kind="ExternalInput")
    r8_d = nc.dram_tensor("r8", [128, 512], BF16, kind="ExternalInput")
    v_d = nc.dram_tensor("v", [B, JM], F32, kind="ExternalOutput")

    cc_in = [nc.dram_tensor(f"cc_in{h}", [128, 512], BF16) for h in range(6)]
    cc_out = [nc.dram_tensor(f"cc_out{h}", [128, 512], BF16, addr_space="Shared")
              for h in range(6)]
    ccw_in = nc.dram_tensor("ccw_in", [16, 16], F32)
    ccw_out = nc.dram_tensor("ccw_out", [16, 16], F32, addr_space="Shared")

    with tile.TileContext(nc) as tc, ExitStack() as ctx:
        _kernel_body(ctx, tc, xblk_d, w_d, s16_d, r8_d, v_d, cc_in, cc_out,
                     ccw_in, ccw_out)
    nc.compile()
    return nc


def _pv(ap, jp=JP):
    """View a [P, jp*64] AP as [P, jp, m, j2]."""
    return ap.rearrange("p (jp m j2) -> p jp m j2", m=N_J, j2=2)


def _kernel_body(ctx, tc, xblk_d, w_d, s16_d, r8_d, v_d, cc_in, cc_out,
                 ccw_in, ccw_out):
    nc = tc.nc
    Act = mybir.ActivationFunctionType
    Alu = mybir.AluOpType
    groups = [list(range(N_CORES))]

    const_pool = ctx.enter_context(tc.tile_pool(name="consts", bufs=1))
    s16 = const_pool.tile([128, 16], BF16)
    nc.sync.dma_start(s16[:], s16_d[:])
    r8 = const_pool.tile([128, 512], BF16)
    nc.sync.dma_start(r8[:], r8_d[:])
    eps_t = const_pool.tile([128, 1], F32)
    nc.vector.memset(eps_t[:], EPS)
    # s64 = 64 * s16 selector, used as the Act-engine zs source
    s64 = const_pool.tile([128, 16], BF16)
    nc.vector.tensor_scalar_mul(s64[:], s16[:], float(CH_J))

    u_pool = ctx.enter_context(tc.tile_pool(name="u", bufs=N_P))
    u_tiles = []

    # small persistent buffers
    sv_pool = ctx.enter_context(tc.tile_pool(name="sv", bufs=1))
    a1_pool = ctx.enter_context(tc.tile_pool(name="a1", bufs=N_P))
    a1_tiles = []

    xall = const_pool.tile([128, N_T * 128], BF16)

    def launch_ar(idx, s_ps, tag):
        """Copy s PSUM -> SBUF bf16 (strip layout), DMA to DRAM, AllReduce."""
        s_sb = sv_pool.tile([128, 512], BF16, tag="s_sb")
        nc.scalar.activation(s_sb[:, :256], s_ps[:, :256], Act.Copy)
        nc.vector.tensor_copy(s_sb[:, 256:], s_ps[:, 256:])
        nc.sync.dma_start(cc_in[idx][:], s_sb[:])
        nc.gpsimd.collective_compute(
            "AllReduce", Alu.add, replica_groups=groups,
            ins=[cc_in[idx][:]], outs=[cc_out[idx][:]])

    def merge_squash(r):
        """Fetch the two AllReduced halves, merge, squash -> v.

        Processed in two column-halves so the downstream consumer (v_rep
        matmul / output DMA) starts as soon as the first half is done."""
        sfa = sv_pool.tile([128, 512], BF16, tag="sfa")
        nc.sync.dma_start(sfa[:], cc_out[2 * r][:])
        sfb = sv_pool.tile([128, 512], BF16, tag="sfb")
        nc.sync.dma_start(sfb[:], cc_out[2 * r + 1][:])
        sf = sv_pool.tile([128, 512], BF16, tag="sf")
        t2 = sv_pool.tile([128, 512], BF16, tag="sfa")  # reuses sfa
        q1 = sv_pool.tile([128, 256], BF16, tag="sfb")
        q2 = sv_pool.tile([128, 128], BF16, tag="s_sb")
        q3 = sv_pool.tile([128, 64], F32, tag="q3")
        q4 = sv_pool.tile([128, 32], F32, tag="q4")
        sq = sv_pool.tile([128, 16], F32, tag="q5")
        rt = sv_pool.tile([128, 16], F32, tag="rt")
        onep = sv_pool.tile([128, 16], F32, tag="onep")
        den = sv_pool.tile([128, 16], F32, tag="den")
        rec = sv_pool.tile([128, 16], F32, tag="rec")
        fs = sv_pool.tile([128, 16], BF16, tag="fs")
        vv = sv_pool.tile([128, 512], F32 if r == 2 else BF16, tag="vv")
        for h in range(2):
            hs = slice(h * 256, (h + 1) * 256)
            nc.vector.tensor_add(sf[:, hs], sfa[:, hs], sfb[:, hs])
            if h == 0:
                nc.scalar.activation(t2[:, hs], sf[:, hs], Act.Square)
            else:
                nc.vector.tensor_mul(t2[:, hs], sf[:, hs], sf[:, hs])
            # tree-reduce over m: [128,(jp,m,j2)] -> [128,(jp,j2)]
            cur, mm = t2[:, hs], N_J
            for buf in (q1, q2, q3, q4, sq):
                n_el = 4 * (mm // 2) * 2
                dst = buf[:, h * n_el:(h + 1) * n_el]
                cv = cur.rearrange("p (jp m j2) -> p jp m j2", m=mm, j2=2)
                nc.vector.tensor_add(
                    dst.rearrange("p (jp m j2) -> p jp m j2",
                                  m=mm // 2, j2=2),
                    cv[:, :, 0:mm // 2, :], cv[:, :, mm // 2:mm, :])
                cur, mm = dst, mm // 2
            js = slice(h * 8, (h + 1) * 8)
            nc.scalar.activation(rt[:, js], sq[:, js], Act.Sqrt,
                                 bias=eps_t[:])
            nc.vector.tensor_scalar_add(onep[:, js], sq[:, js], 1.0)
            nc.vector.tensor_mul(den[:, js], rt[:, js], onep[:, js])
            nc.vector.reciprocal(rec[:, js], den[:, js])
            nc.vector.tensor_mul(fs[:, js], sq[:, js], rec[:, js])
            # v = sf * fs  (fs broadcast over m via stride-0 AP)
            fsb = fs[:, js].rearrange("p (jp j2) -> p jp j2", j2=2) \
                .unsqueeze(2).broadcast_to((128, 4, N_J, 2))
            nc.vector.tensor_mul(_pv(vv[:, hs], 4),
                                 _pv(sf[:, hs], 4), fsb)
            if r == 2:
                for q in range(4):
                    nc.sync.dma_start(
                        v_d[:, q * 512 + h * 256:q * 512 + (h + 1) * 256],
                        vv[32 * q:32 * q + 16, hs])
        return vv

    # ---------------- Phase A: produce u, accumulate s0 ----------------
    # Software-pipelined: s0 matmuls for tile t are issued during tile t+1.
    # s0 is accumulated in two tile-halves; the first half's AllReduce
    # overlaps the second half of the loop.
    with tc.tile_pool(name="wbuf", bufs=3) as w_pool, \
         tc.tile_pool(name="uprod", bufs=2, space="PSUM") as up_pool, \
         tc.tile_pool(name="s0ps", bufs=1, space="PSUM") as s0_pool:
        s0_ps = s0_pool.tile([128, 512], F32)
        nc.vector.memset(s0_ps[:], 0.0)

        def issue_s0(t):
            ut = u_tiles[t // 2]
            f0 = (t % 2) * JM
            for q in range(4):
                o0 = q * 512
                nc.tensor.matmul(
                    s0_ps[32 * q:32 * q + 16, :], s16[:],
                    ut[:, f0 + o0:f0 + o0 + 512],
                    start=(t % H_T == 0), stop=(t % H_T == H_T - 1),
                    tile_position=(0, 32 * q))
            if t == H_T - 1:
                launch_ar(0, s0_ps, "a")

        for t in range(N_T):
            wt = w_pool.tile([128, JM], BF16)
            r0 = t * 128
            nc.sync.dma_start(wt[:, :1024], w_d[r0:r0 + 128, :1024])
            nc.sync.dma_start(wt[:, 1024:], w_d[r0:r0 + 128, 1024:])
            if t == 0:
                # xblk prefetch (host packs it partition-major: one fast
                # contiguous DMA), then a warmup collective (the first real
                # AllReduce otherwise pays ~30us of collective-stack warmup)
                nc.sync.dma_start(xall[:], xblk_d[:])
                warm = const_pool.tile([16, 16], F32)
                nc.vector.memset(warm[:], 0.0)
                nc.sync.dma_start(ccw_in[:], warm[:])
                nc.gpsimd.collective_compute(
                    "AllReduce", Alu.add, replica_groups=groups,
                    ins=[ccw_in[:]], outs=[ccw_out[:]])
            xt = xall[:, t * 128:(t + 1) * 128]
            if t % 2 == 0:
                upair = u_pool.tile([128, 2 * JM], BF16)
                u_tiles.append(upair)
            f0 = (t % 2) * JM
            ups = []
            for h in range(2):
                up = up_pool.tile([128, 1024], F32)
                ups.append(up)
                for q in range(2):
                    nc.tensor.matmul(
                        up[:, q * 512:(q + 1) * 512], xt,
                        wt[:, h * 1024 + q * 512: h * 1024 + (q + 1) * 512],
                        start=True, stop=True)
            # copies split across Scalar and Vector engines
            nc.scalar.activation(upair[:, f0:f0 + 1024], ups[0][:], Act.Copy)
            nc.vector.tensor_copy(upair[:, f0 + 1024:f0 + 2048], ups[1][:])
            if t > 0:
                issue_s0(t - 1)
        issue_s0(N_T - 1)
        launch_ar(1, s0_ps, "b")

    v_sb = merge_squash(0)

    # ---------------- Phases C: routing passes r=1,2 ----------------
    # Tiles processed in pairs: all big DVE ops span two tiles (the pair's
    # 64 j-slots are treated as a 64-wide "jp" axis).
    for r in (1, 2):
        with tc.tile_pool(name=f"vr{r}", bufs=1) as vr_pool, \
             tc.tile_pool(name=f"scr{r}", bufs=1) as scr_pool, \
             tc.tile_pool(name=f"uvb{r}", bufs=1) as uv_pool, \
             tc.tile_pool(name=f"tr{r}", bufs=1) as tr_pool, \
             tc.tile_pool(name=f"sm{r}", bufs=3) as sm_pool, \
             tc.tile_pool(name=f"vrps{r}", bufs=2, space="PSUM") as vr_ps_pool, \
             tc.tile_pool(name=f"sps{r}", bufs=1, space="PSUM") as s_ps_pool:
            # v_rep (128, JM) bf16: broadcast v over the 8 i-slots
            v_rep = vr_pool.tile([128, JM], BF16)
            for h in range(2):
                vp = vr_ps_pool.tile([128, 1024], F32)
                for q in range(2):
                    qq = 2 * h + q
                    nc.tensor.matmul(
                        vp[:, q * 512:(q + 1) * 512],
                        r8[:, qq * 128:(qq + 1) * 128], v_sb[:],
                        start=True, stop=True)
                if h == 0:
                    nc.scalar.activation(v_rep[:, :1024], vp[:], Act.Copy)
                else:
                    nc.vector.tensor_copy(v_rep[:, 1024:], vp[:])
            s_ps = s_ps_pool.tile([128, 512], F32)
            nc.vector.memset(s_ps[:], 0.0)

            pend = [None]

            def issue_tail(p, expt, se, uv_old, r=r, s_ps=s_ps):
                """zs/cu + s-matmuls for pair p (issued one pair late).

                cu is written into pair p's retired uv buffer (uv is dead
                after the first tree level)."""
                upair = u_tiles[p]
                rs = sm_pool.tile([128, 2], F32, tag="rs")
                nc.vector.reciprocal(rs[:], se[:])
                cu = scr_pool.tile([128, 2 * JM], BF16)
                ctb = expt[:].rearrange("p (jp j2) -> p jp j2", j2=2) \
                    .unsqueeze(2).broadcast_to((128, 2 * JP, N_J, 2))
                nc.vector.tensor_mul(
                    _pv(cu[:], 2 * JP), _pv(upair[:], 2 * JP), ctb)
                for half in range(2):
                    # zs = 64/Z on the Scalar engine (s64 * rs)
                    zs = sm_pool.tile([128, 16], BF16, tag=f"zs{half}")
                    nc.scalar.activation(zs[:], s64[:], Act.Copy,
                                         scale=rs[:, half:half + 1])
                    t = 2 * p + half
                    f0 = half * JM
                    for q in range(4):
                        o0 = q * 512
                        nc.tensor.matmul(
                            s_ps[32 * q:32 * q + 16, :], zs[:],
                            cu[:, f0 + o0:f0 + o0 + 512],
                            start=(t % H_T == 0), stop=(t % H_T == H_T - 1),
                            tile_position=(0, 32 * q))
                    if t == H_T - 1:
                        launch_ar(2 * r, s_ps, "a")

            vrb = v_rep[:].unsqueeze(1).broadcast_to((128, 2, JM))
            for p in range(N_P):
                upair = u_tiles[p]
                uv = uv_pool.tile([128, 2 * JM], BF16, tag="uv")
                if p == 0:
                    # first pair: split on v_rep col-halves so it can start
                    # as soon as the first v_rep copy lands
                    for h in range(2):
                        hs = slice(h * 1024, (h + 1) * 1024)
                        nc.vector.tensor_mul(
                            uv[:].rearrange("p (q f) -> p q f", q=2)[:, :, hs],
                            upair[:].rearrange("p (q f) -> p q f", q=2)
                            [:, :, hs],
                            v_rep[:, hs].unsqueeze(1).broadcast_to(
                                (128, 2, 1024)))
                else:
                    nc.vector.tensor_mul(
                        uv[:].rearrange("p (q f) -> p q f", q=2),
                        upair[:].rearrange("p (q f) -> p q f", q=2), vrb)
                # m-reduce tree over both tiles: 32 -> 16 -> 8 -> 4 -> 2 -> 1
                # ping-pong between two scratch tiles (address subranges)
                tA = tr_pool.tile([128, 2 * JM // 2], BF16, tag="tA")
                tB = tr_pool.tile([128, 2 * JM // 4], BF16, tag="tB")
                cur, mm = uv[:], N_J
                for lvl in range(4):
                    n_el = 2 * JP * (mm // 2) * 2
                    dst = (tA if lvl % 2 == 0 else tB)[:, :n_el]
                    cv = cur.rearrange("p (jp m j2) -> p jp m j2",
                                       m=mm, j2=2)
                    nc.vector.tensor_add(
                        dst.rearrange("p (jp m j2) -> p jp m j2",
                                      m=mm // 2, j2=2),
                        cv[:, :, 0:mm // 2, :], cv[:, :, mm // 2:mm, :])
                    cur, mm = dst, mm // 2
                l4v = cur.rearrange("p (jp m j2) -> p jp m j2", m=2, j2=2)
                if r == 1:
                    a1 = a1_pool.tile([128, 2 * CH_J], BF16)
                    a1_tiles.append(a1)
                    nc.vector.tensor_add(
                        a1[:].rearrange("p (jp m j2) -> p jp m j2",
                                        m=1, j2=2),
                        l4v[:, :, 0:1, :], l4v[:, :, 1:2, :])
                    logits = a1
                else:
                    a2 = sm_pool.tile([128, 2 * CH_J], BF16, tag="a2")
                    nc.vector.tensor_add(
                        a2[:].rearrange("p (jp m j2) -> p jp m j2",
                                        m=1, j2=2),
                        l4v[:, :, 0:1, :], l4v[:, :, 1:2, :])
                    lg = sm_pool.tile([128, 2 * CH_J], BF16, tag="lg")
                    nc.vector.tensor_add(lg[:], a2[:], a1_tiles[p][:])
                    logits = lg
                expt = sm_pool.tile([128, 2 * CH_J], BF16, tag="expt")
                se = sm_pool.tile([128, 2], F32, tag="se")
                for half in range(2):
                    nc.scalar.activation(
                        expt[:, half * CH_J:(half + 1) * CH_J],
                        logits[:, half * CH_J:(half + 1) * CH_J],
                        Act.Exp, scale=8.0,
                        accum_out=se[:, half:half + 1])
                if pend[0] is not None:
                    issue_tail(*pend[0])
                pend[0] = (p, expt, se, uv)
            issue_tail(*pend[0])
            launch_ar(2 * r + 1, s_ps, "b")
        v_sb = merge_squash(r)


def _host_inputs(inputs, w):
    """Build per-core input maps (host-side shard + block-diag pack).

    Free-dim layout for w/u/s/v on-device is paired: f = jp*64 + m*2 + j2
    with j = 2*jp + j2.
    """
    import ml_dtypes
    x = np.asarray(inputs, dtype=np.float32)
    w = np.asarray(w, dtype=np.float32)
    s16 = np.zeros((128, 16), dtype=np.float32)
    for i8 in range(8):
        for b in range(16):
            s16[i8 * 16 + b, b] = 1.0
    s16 = s16.astype(ml_dtypes.bfloat16)
    r8 = np.zeros((128, 512), dtype=np.float32)
    for q in range(4):
        for i8 in range(8):
            for b in range(16):
                r8[32 * q + b, q * 128 + i8 * 16 + b] = 1.0
    r8 = r8.astype(ml_dtypes.bfloat16)
    in_maps = []
    for k in range(N_CORES):
        i0 = k * I_LOC
        # (256, 16, 64, 32) -> (256*16, jp, j2, m) -> (.., jp, m, j2)
        wk = w[i0:i0 + I_LOC].reshape(I_LOC * N_I, JP, 2, N_J)
        wk = np.ascontiguousarray(wk.transpose(0, 1, 3, 2)).reshape(
            I_LOC * N_I, JM).astype(ml_dtypes.bfloat16)
        xk = x[:, i0:i0 + I_LOC, :]  # (B, 256, 16)
        xblk = np.zeros((N_T, 128, 128), dtype=np.float32)
        # xblk[t, i8*16+n, i8*16+b] = x[b, i0+8t+i8, n]
        xv = xk.transpose(1, 2, 0).reshape(N_T, 8, N_I, B)  # (t, i8, n, b)
        for i8 in range(8):
            xblk[:, i8 * 16:i8 * 16 + N_I, i8 * 16:i8 * 16 + B] = xv[:, i8]
        # partition-major so the device loads it as one contiguous DMA
        xblk = np.ascontiguousarray(
            xblk.transpose(1, 0, 2)).reshape(128, N_T * 128)
        xblk = xblk.astype(ml_dtypes.bfloat16)
        in_maps.append({"xblk": xblk, "w": wk, "s16": s16, "r8": r8})
    return in_maps


def kernel(inputs, w, _trace=False):
    key = "nc"
    if key not in _CACHE:
        _CACHE[key] = _build_program()
    nc = _CACHE[key]
    in_maps = _host_inputs(inputs, w)
    res = run_bass_kernel_spmd(nc, in_maps, list(range(N_CORES)),
                               trace=_trace)
    vp = res.results[0]["v"].reshape(B, JP, N_J, 2)  # (b, jp, m, j2)
    v = np.ascontiguousarray(vp.transpose(0, 1, 3, 2)).reshape(
        B, CH_J, N_J).astype(np.float32)
    if _trace:
        kernel._last = res
    return v
